# revision 1
# baseline (speedup 1.0000x reference)
"""DepthDC fused kernel for 8 Trainium2 NeuronCores.

Reference computation (N=2, C=64, H=W=256, d=2):
  patches[n,c,k,h,w] = xpad[n,c,h+ki*d, w+kj*d]   (k=3*ki+kj, pad d)
  out1 = sum_k patches * y.reshape(N,C,9,H,W)
  out  = leaky_relu(conv3x3(out1, fuse_w) + fuse_b, 0.2)

Sharding: 8 cores = batch(2) x H-quarters(4). Each core produces a
[64, 64, 256] output slab. Host restages the inputs per core so every
device DMA moves one fully contiguous block per SBUF partition (one
descriptor per partition), and the streamed tensors are bf16 to halve
HBM traffic. DMAs alternate between the two HWDGE rings (SP / ACT) so
transfers overlap on hardware.

Per-core layout: the 64 output rows split into two 32-row halves mapped
to SBUF partition halves (partition = c + 64*s). All engines see uniform
[128, F] tiles:
  - DVE: 9 elementwise products prod_k = x_shift(k) * y_k        (bf16)
  - PE:  k-reduction via identity matmul, accumulated in PSUM    (bf16)
  - PE:  3x3 dense conv as 9 accumulating matmuls over C=64      (bf16)
  - ACT: PSUM->SBUF copies and the bias-add of the epilogue
  - DVE: leaky_relu(v) = max(v, 0.2*v) final combine
Work is streamed over 4-row h-chunks with double-buffered y DMA.
"""

import sys

sys.path.insert(0, "/opt/trn_rl_repo")

import numpy as np

import concourse.bass as bass
import concourse.mybir as mybir
import concourse.tile as tile
from concourse import bacc
from concourse.bass_utils import run_bass_kernel_spmd

F32 = mybir.dt.float32
BF16 = mybir.dt.bfloat16
NPBF16 = mybir.dt.np(BF16)
AF = mybir.ActivationFunctionType

N, C, H, W = 2, 64, 256, 256
D = 2  # dilation == pad
NEG_SLOPE = 0.2
NCORES = 8
HB = 64          # output rows per core
HH = 32          # output rows per half
Q = 34           # out1 rows per half (HH + 2 conv halo)
XR = Q + 4       # x rows per half block (38)
XW = W + 2 * D   # padded x width (260)
OW = W + 2       # padded out1 width (258)
RC = 4           # rows per chunk
NCH = 9          # reduce chunks per half: 8 x 4 rows + 1 x 2 rows
NCONV = 8        # conv chunks per half: 8 x 4 rows
YROW = [RC * c for c in range(NCH)]           # chunk start rows
YRC = [min(RC, Q - r) for r in YROW]          # chunk row counts
YLEN = 9 * Q * W                              # yh elems per partition


def _build_program(loop_reps=None, epilogue="split", mode="full",
                   merged_k=False):
    nc = bacc.Bacc("TRN2", target_bir_lowering=False, debug=False,
                   num_devices=NCORES)

    xh_d = nc.dram_tensor("xh", [128, XR, XW], BF16, kind="ExternalInput").ap()
    yh_d = nc.dram_tensor("yh", [128, YLEN], BF16, kind="ExternalInput").ap()
    wt_d = nc.dram_tensor("wt", [128, 9, 128], BF16, kind="ExternalInput").ap()
    id_d = nc.dram_tensor("ident", [128, 128], BF16,
                          kind="ExternalInput").ap()
    b_d = nc.dram_tensor("bias", [128, 1], F32, kind="ExternalInput").ap()
    out_d = nc.dram_tensor("out", [NCONV, 128, RC, W], BF16,
                           kind="ExternalOutput").ap()

    with tile.TileContext(nc) as tc:
        from contextlib import ExitStack
        with ExitStack() as ctx:
            const = ctx.enter_context(tc.tile_pool(name="const", bufs=1))
            y_pool = ctx.enter_context(tc.tile_pool(name="y_pool", bufs=2))
            yt_pool = ctx.enter_context(tc.tile_pool(name="yt_pool", bufs=1))
            p_pool = ctx.enter_context(
                tc.tile_pool(name="p_pool", bufs=2 if merged_k else 6))
            o_pool = ctx.enter_context(tc.tile_pool(name="o_pool", bufs=3))
            v_pool = ctx.enter_context(tc.tile_pool(name="v_pool", bufs=3))
            ps1_pool = ctx.enter_context(
                tc.tile_pool(name="ps1_pool", bufs=2, space="PSUM"))
            ps2_pool = ctx.enter_context(
                tc.tile_pool(name="ps2_pool", bufs=2, space="PSUM"))

            # constants / whole-slab x / whole-slab out1
            w_sb = const.tile([128, 9, 128], BF16, name="w_sb")
            nc.sync.dma_start(w_sb[:], wt_d)
            id_sb = const.tile([128, 128], BF16, name="id_sb")
            nc.sync.dma_start(id_sb[:], id_d)
            b_sb = const.tile([128, 1], F32, name="b_sb")
            nc.sync.dma_start(b_sb[:], b_d)
            x_sb = const.tile([128, XR, XW], BF16, name="x_sb")
            nc.scalar.dma_start(x_sb[:], xh_d)
            o1_sb = const.tile([128, Q, OW], BF16, name="o1_sb")
            # zero the conv W-padding columns once (exact bit pattern;
            # an ALU 0*garbage would propagate NaN payloads on HW)
            nc.vector.memset(o1_sb[:, :, 0:1], 0.0)
            nc.vector.memset(o1_sb[:, :, OW - 1:OW], 0.0)
            # Wait-merge scratch: one cheap DVE copy per input DMA converts
            # DMA-completion semaphores into DVE program order, so compute
            # instructions never need more than 1 foreign wait sem (the
            # TT-struct wait-slot limit in walrus codegen is tight).
            scr = const.tile([128, 8], BF16, name="scr")
            nc.vector.tensor_copy(scr[:, 0:1], x_sb[:, 0, 0:1])
            nc.vector.tensor_copy(scr[:, 1:2], x_sb[:, XR - 1, 0:1])
            nc.vector.tensor_copy(scr[:, 2:3], w_sb[:, 0, 0:1])
            nc.vector.tensor_copy(scr[:, 3:4], id_sb[:, 0:1])
            nc.vector.tensor_copy(scr[:, 4:5],
                                  b_sb[:, 0:1].bitcast(BF16)[:, 0:1])

            def load_pair(pb):
                # chunks (2pb, 2pb+1) are adjacent in the flat yh layout:
                # one DMA, one ~37KB descriptor per partition
                q0 = YROW[2 * pb]
                nrow = YRC[2 * pb] + (YRC[2 * pb + 1]
                                      if 2 * pb + 1 < NCH else 0)
                off = 9 * W * q0
                if nrow == 2 * RC:
                    y_t = y_pool.tile([128, 2, 9, RC, W], BF16, name="y_t",
                                      tag="y_t")
                    src = yh_d[:, off:off + 9 * nrow * W].rearrange(
                        "p (i k r w) -> p i k r w", i=2, k=9, r=RC)
                else:
                    y_t = yt_pool.tile([128, 1, 9, nrow, W], BF16,
                                       name="y_tl", tag="y_tl")
                    src = yh_d[:, off:off + 9 * nrow * W].rearrange(
                        "p (i k r w) -> p i k r w", i=1, k=9, r=nrow)
                nc.sync.dma_start(y_t[:], src)
                nc.vector.tensor_copy(scr[:, 5:6], y_t[:, 0, 0, 0, 0:1])
                return y_t

            def reduce_chunk(cb, y_t):
                q0 = YROW[cb]
                rc = YRC[cb]
                yv = y_t[:, cb % 2]
                if mode == "dma":
                    return
                ps1 = ps1_pool.tile([128, RC, W], F32, name="ps1", tag="ps1")
                if merged_k:
                    # all 9 products in one tile; one matmul per row pair
                    # with a stride-0 (broadcast) PSUM out: the per-element
                    # has_written bit accumulates the 9 k-planes in-place
                    p_f = p_pool.tile([128, 9, RC, W], BF16, name="p_f",
                                      tag="p_f")
                    for k in range(9):
                        ki, kj = divmod(k, 3)
                        x_view = x_sb[:, q0 + 2 * ki: q0 + 2 * ki + rc,
                                      2 * kj: 2 * kj + W]
                        nc.vector.tensor_mul(p_f[:, k, 0:rc], x_view,
                                             yv[:, k, 0:rc])
                    for j2 in range(rc // 2):
                        r0, r1 = 2 * j2, 2 * j2 + 2
                        out_b = ps1[:, r0:r1, :].rearrange(
                            "p (o a) b -> p o a b", o=1).broadcast_to(
                            [128, 9, 2, W])
                        nc.tensor.matmul(out_b, lhsT=id_sb[:],
                                         rhs=p_f[:, :, r0:r1, :],
                                         start=True, stop=True)
                else:
                    for k in range(9):
                        ki, kj = divmod(k, 3)
                        p_t = p_pool.tile([128, RC, W], BF16, name="p_t",
                                          tag="p_t")
                        x_view = x_sb[:, q0 + 2 * ki: q0 + 2 * ki + rc,
                                      2 * kj: 2 * kj + W]
                        nc.vector.tensor_mul(p_t[:, 0:rc], x_view,
                                             yv[:, k, 0:rc])
                        for j2 in range(rc // 2):
                            r0, r1 = 2 * j2, 2 * j2 + 2
                            nc.tensor.matmul(
                                ps1[:, r0:r1, :], lhsT=id_sb[:],
                                rhs=p_t[:, r0:r1, :],
                                start=(k == 0), stop=(k == 8))
                nc.scalar.copy(o1_sb[:, q0:q0 + rc, 1:W + 1], ps1[:, 0:rc])

            def conv_chunk(j):
                if mode in ("dma", "noconv"):
                    return
                m0 = RC * j
                ps2 = ps2_pool.tile([128, RC, W], F32, name="ps2", tag="ps2")
                for t in range(9):
                    i3, j3 = divmod(t, 3)
                    for j2 in (0, 1):
                        r0 = 2 * j2
                        nc.tensor.matmul(
                            ps2[:, r0:r0 + 2, :], lhsT=w_sb[:, t],
                            rhs=o1_sb[:, m0 + i3 + r0: m0 + i3 + r0 + 2,
                                      j3: j3 + W],
                            start=(t == 0), stop=(t == 8))
                o_t = o_pool.tile([128, RC, W], BF16, name="o_t", tag="o_t")
                if epilogue == "lrelu":
                    nc.scalar.activation(o_t[:], ps2[:], AF.Lrelu,
                                         bias=b_sb[:, 0:1], scale=1.0,
                                         alpha=NEG_SLOPE)
                else:
                    # v = ps2 + bias (ACT), then leaky = max(v, 0.2v) (DVE)
                    v_t = v_pool.tile([128, RC, W], F32, name="v_t",
                                      tag="v_t")
                    nc.scalar.activation(v_t[:], ps2[:], AF.Identity,
                                         bias=b_sb[:, 0:1], scale=1.0)
                    nc.vector.scalar_tensor_tensor(
                        o_t[:], v_t[:], NEG_SLOPE, v_t[:],
                        mybir.AluOpType.mult, mybir.AluOpType.max)
                nc.scalar.dma_start(out_d[j], o_t[:])

            def body():
                y_cur = load_pair(0)
                y_nxt = load_pair(1)
                for cb in range(NCH):
                    reduce_chunk(cb, y_cur)
                    if cb >= 1:
                        conv_chunk(cb - 1)
                    if cb % 2 == 1:
                        y_cur = y_nxt
                        pb = (cb + 1) // 2 + 1
                        if pb * 2 < NCH:
                            y_nxt = load_pair(pb)

            if loop_reps is None:
                body()
            else:
                with tc.For_i(0, loop_reps, 1,
                              hint_engines=(mybir.EngineType.PE,)):
                    body()

    nc.compile()
    return nc


_PROGRAM = None


def _get_program():
    global _PROGRAM
    if _PROGRAM is None:
        _PROGRAM = _build_program()
    return _PROGRAM


def make_in_maps(x, y, fuse_w, fuse_b):
    x = np.asarray(x, dtype=np.float32)
    y = np.asarray(y, dtype=np.float32)
    fuse_w = np.asarray(fuse_w, dtype=np.float32)
    fuse_b = np.asarray(fuse_b, dtype=np.float32)

    # block-diagonal conv weights: each partition half (h-half of the
    # slab) contracts with its own copy of W_tap in one K=128 matmul
    wt = np.zeros((9, 128, 128), np.float32)
    for t in range(9):
        i, j = divmod(t, 3)
        wtap = fuse_w[:, :, i, j].T  # [c_in, c_out]
        wt[t, 0:64, 0:64] = wtap
        wt[t, 64:128, 64:128] = wtap
    wt = np.ascontiguousarray(wt.transpose(1, 0, 2)).astype(NPBF16)
    ident = np.eye(128, dtype=np.float32).astype(NPBF16)
    bias = np.concatenate([fuse_b, fuse_b]).astype(np.float32)[:, None]

    y5 = y.reshape(N, C, 9, H, W)
    in_maps = []
    for core in range(NCORES):
        n, hb = divmod(core, 4)
        h0 = hb * HB
        # x: [128, XR, XW] bf16, partition = c + 64*s
        xh = np.zeros((2, C, XR, XW), np.float32)
        for s in (0, 1):
            r0 = h0 + HH * s - 3
            lo, hi = max(r0, 0), min(r0 + XR, H)
            xh[s, :, lo - r0:hi - r0, D:D + W] = x[n, :, lo:hi, :]
        xh = xh.reshape(128, XR, XW).astype(NPBF16)
        # y: flat [128, YLEN] bf16; chunk cb occupies the contiguous
        # block [9*W*YROW[cb] : +9*rc*W) per partition, laid out [k,r,w]
        y34 = np.zeros((2, C, 9, Q, W), np.float32)
        for s in (0, 1):
            r0 = h0 + HH * s - 1
            lo, hi = max(r0, 0), min(r0 + Q, H)
            y34[s, :, :, lo - r0:hi - r0, :] = y5[n, :, :, lo:hi, :]
        yh = np.empty((128, YLEN), NPBF16)
        for cb in range(NCH):
            q0, rc = YROW[cb], YRC[cb]
            off = 9 * W * q0
            blk = y34[:, :, :, q0:q0 + rc, :].reshape(128, 9 * rc * W)
            yh[:, off:off + 9 * rc * W] = blk.astype(NPBF16)
        in_maps.append({"xh": xh, "yh": yh, "wt": wt, "ident": ident,
                        "bias": bias})
    return in_maps


def gather_out(res):
    out = np.empty((N, C, H, W), np.float32)
    for core in range(NCORES):
        n, hb = divmod(core, 4)
        o = np.asarray(res.results[core]["out"]).astype(np.float32)
        o = o.reshape(NCONV, 2, C, RC, W).transpose(2, 1, 0, 3, 4)
        out[n, :, hb * HB:(hb + 1) * HB, :] = o.reshape(C, HB, W)
    return out


def run(x, y, fuse_w, fuse_b, trace=False, **kw):
    nc = _get_program()
    in_maps = make_in_maps(x, y, fuse_w, fuse_b)
    res = run_bass_kernel_spmd(nc, in_maps, list(range(NCORES)),
                               trace=trace, **kw)
    return gather_out(res), res


def kernel(x, y, fuse_w, fuse_b):
    out, _ = run(x, y, fuse_w, fuse_b, trace=False)
    return out

